# revision 12
# baseline (speedup 1.0000x reference)
"""Trainium2 Bass kernel for nn_ExampleLabelWeights (segment_reduce).

Computes: gather per-example weight rows + cardinality, masked softmax over
each row's valid slots, weighted sum of losses, global scalar sum.

Strategy (8 NeuronCores, data-parallel over the batch):
  - batch rows (131072) are split 16384/core.
  - the params table is packed host-side into 20-float rows
    [w0..w15, cardinality_f32, pad, pad, pad] (80B, 16B aligned) and
    replicated to every core, so ONE indirect-DMA descriptor per batch row
    fetches both the weights and the cardinality.
  - on-device per core: indirect gather (GPSIMD SWDGE) -> exp on ACT ->
    mask build + masked reduces + reciprocal on DVE -> per-row ratio ->
    per-core scalar via PE matmul with ones.
  - host sums the 8 per-core partials (the only cross-core reduction).

Written in raw bass (explicit engine programs + semaphores): the walrus
build in this container only supports ONE sync-wait command per
instruction, which TileContext's auto-generated semaphores violate.
"""

from contextlib import ExitStack

import numpy as np

import concourse.bass as bass
import concourse.mybir as mybir
from concourse.bass_utils import run_bass_kernel_spmd

F32 = mybir.dt.float32
I32 = mybir.dt.int32

NCORES = 8
B = 131072
MAXC = 16
V = 1_000_000
W = 20                 # packed table row width (16 weights + card + 3 pad)
P = 128                # SBUF partitions
BC = B // NCORES       # rows per core
COLS = BC // P         # row-groups per partition (128)
CHUNKS = 4
CC = COLS // CHUNKS    # row-groups per chunk per partition


def build_kernel(chunks: int = CHUNKS):
    cc = COLS // chunks
    nc = bass.Bass()
    ptab = nc.declare_dram_parameter("ptab", [V, W], F32, isOutput=False)
    idx = nc.declare_dram_parameter("idx", [P, COLS], I32, isOutput=False)
    losses = nc.declare_dram_parameter("losses", [P, COLS * MAXC], F32,
                                       isOutput=False)
    out = nc.declare_dram_parameter("out", [1, 1], F32, isOutput=True)

    with ExitStack() as ctx:
        sem_idx = ctx.enter_context(nc.semaphore("sem_idx"))
        sem_iota = ctx.enter_context(nc.semaphore("sem_iota"))
        sem_g = [ctx.enter_context(nc.semaphore(f"sem_g{k}"))
                 for k in range(chunks)]
        sem_l = [ctx.enter_context(nc.semaphore(f"sem_l{k}"))
                 for k in range(chunks)]
        sem_exp = ctx.enter_context(nc.semaphore("sem_exp"))
        sem_dve = ctx.enter_context(nc.semaphore("sem_dve"))
        sem_mm = ctx.enter_context(nc.semaphore("sem_mm"))
        sem_res = ctx.enter_context(nc.semaphore("sem_res"))
        sem_out = ctx.enter_context(nc.semaphore("sem_out"))
        all_sems = [sem_idx, sem_iota, *sem_g, *sem_l, sem_exp,
                    sem_dve, sem_mm, sem_res, sem_out]

        idxt = ctx.enter_context(nc.sbuf_tensor("idxt", [P, COLS], I32))
        ioti = ctx.enter_context(nc.sbuf_tensor("ioti", [P, cc * MAXC], I32))
        iotat = ctx.enter_context(nc.sbuf_tensor("iotat", [P, cc * MAXC], F32))
        ones = ctx.enter_context(nc.sbuf_tensor("ones", [P, 1], F32))
        acc = ctx.enter_context(nc.sbuf_tensor("acc", [P, COLS], F32))
        colsum = ctx.enter_context(nc.sbuf_tensor("colsum", [P, 1], F32))
        res = ctx.enter_context(nc.sbuf_tensor("res", [1, 1], F32))
        tot = ctx.enter_context(nc.psum_tensor("tot", [1, 1], F32))

        pk, lk, ek, mk, em, nm, cardt, den, num, rd = ([] for _ in range(10))
        for k in range(chunks):
            pk.append(ctx.enter_context(
                nc.sbuf_tensor(f"pk{k}", [P, cc * W], F32)))
            lk.append(ctx.enter_context(
                nc.sbuf_tensor(f"lk{k}", [P, cc * MAXC], F32)))
            ek.append(ctx.enter_context(
                nc.sbuf_tensor(f"ek{k}", [P, cc * MAXC], F32)))
            mk.append(ctx.enter_context(
                nc.sbuf_tensor(f"mk{k}", [P, cc * MAXC], F32)))
            em.append(ctx.enter_context(
                nc.sbuf_tensor(f"em{k}", [P, cc * MAXC], F32)))
            nm.append(ctx.enter_context(
                nc.sbuf_tensor(f"nm{k}", [P, cc * MAXC], F32)))
            cardt.append(ctx.enter_context(
                nc.sbuf_tensor(f"cardt{k}", [P, cc], F32)))
            den.append(ctx.enter_context(
                nc.sbuf_tensor(f"den{k}", [P, cc], F32)))
            num.append(ctx.enter_context(
                nc.sbuf_tensor(f"num{k}", [P, cc], F32)))
            rd.append(ctx.enter_context(
                nc.sbuf_tensor(f"rd{k}", [P, cc], F32)))

        def r3(ap, width):
            return ap.rearrange("p (c u) -> p c u", u=width)

        with nc.Block() as block:

            @block.sync
            def _(sync):
                sync.dma_start(out=idxt[:, :], in_=idx[:, :]).then_inc(
                    sem_idx, 16)
                for k in range(chunks):
                    sync.dma_start(
                        out=lk[k][:, :],
                        in_=losses[:, k * cc * MAXC:(k + 1) * cc * MAXC],
                    ).then_inc(sem_l[k], 16)
                sync.wait_ge(sem_res, 1)
                sync.dma_start(out=out[:, :], in_=res[:, :]).then_inc(
                    sem_out, 16)
                sync.wait_ge(sem_out, 16)

            @block.gpsimd
            def _(gpsimd):
                gpsimd.iota(
                    ioti[:, :], pattern=[[0, cc], [1, MAXC]],
                    base=0, channel_multiplier=0,
                ).then_inc(sem_iota, 1)
                gpsimd.wait_ge(sem_idx, 16)
                for k in range(chunks):
                    gpsimd.indirect_dma_start(
                        out=pk[k][:, :],
                        out_offset=None,
                        in_=ptab[:, :],
                        in_offset=bass.IndirectOffsetOnAxis(
                            ap=idxt[:, k * cc:(k + 1) * cc], axis=0
                        ),
                    ).then_inc(sem_g[k], 16)

            @block.scalar
            def _(scalar):
                for k in range(chunks):
                    scalar.wait_ge(sem_g[k], 16)
                    scalar.activation(
                        out=r3(ek[k][:, :], MAXC)[:, :, :],
                        in_=r3(pk[k][:, :], W)[:, :, 0:MAXC],
                        func=mybir.ActivationFunctionType.Exp,
                    ).then_inc(sem_exp, 1)

            # The DVE pipeline does not interlock same-engine RAW hazards:
            # every dependent pair needs an explicit wait on the engine's
            # completion counter. Track producer indices at build time and
            # emit monotone wait_ge's (skipping already-covered thresholds).
            marks = {}

            @block.vector
            def _(vector):
                state = {"n": 0, "hw": 0}

                def bump(inst):
                    state["n"] += 1
                    inst.then_inc(sem_dve, 1)
                    return state["n"]

                def dep(*ths):
                    th = max(ths)
                    if th > state["hw"]:
                        vector.wait_ge(sem_dve, th)
                        state["hw"] = th

                vector.wait_ge(sem_iota, 1)
                i_iotat = bump(vector.tensor_copy(out=iotat[:, :],
                                                  in_=ioti[:, :]))
                bump(vector.memset(ones[:, :], 1.0))
                for k in range(chunks):
                    vector.wait_ge(sem_g[k], 16)
                    i_card = bump(vector.tensor_copy(
                        out=r3(cardt[k][:, :], 1)[:, :, :],
                        in_=r3(pk[k][:, :], W)[:, :, MAXC:MAXC + 1],
                    ))
                    dep(i_card, i_iotat)
                    i_mk = bump(vector.tensor_tensor(
                        out=r3(mk[k][:, :], MAXC)[:, :, :],
                        in0=r3(cardt[k][:, :], 1).broadcast_to([P, cc, MAXC]),
                        in1=r3(iotat[:, :], MAXC)[:, :, :],
                        op=mybir.AluOpType.is_gt,
                    ))
                    vector.wait_ge(sem_exp, k + 1)
                    dep(i_mk)
                    i_em = bump(vector.tensor_tensor(
                        out=em[k][:, :], in0=ek[k][:, :], in1=mk[k][:, :],
                        op=mybir.AluOpType.mult,
                    ))
                    dep(i_em)
                    i_den = bump(vector.tensor_reduce(
                        out=den[k][:, :], in_=r3(em[k][:, :], MAXC)[:, :, :],
                        axis=mybir.AxisListType.X, op=mybir.AluOpType.add,
                    ))
                    vector.wait_ge(sem_l[k], 16)
                    i_nm = bump(vector.tensor_tensor(
                        out=nm[k][:, :], in0=em[k][:, :], in1=lk[k][:, :],
                        op=mybir.AluOpType.mult,
                    ))
                    dep(i_nm)
                    i_num = bump(vector.tensor_reduce(
                        out=num[k][:, :], in_=r3(nm[k][:, :], MAXC)[:, :, :],
                        axis=mybir.AxisListType.X, op=mybir.AluOpType.add,
                    ))
                    dep(i_den)
                    i_rd = bump(vector.reciprocal(out=rd[k][:, :],
                                                  in_=den[k][:, :]))
                    dep(i_num, i_rd)
                    bump(vector.tensor_tensor(
                        out=acc[:, k * cc:(k + 1) * cc],
                        in0=num[k][:, :], in1=rd[k][:, :],
                        op=mybir.AluOpType.mult,
                    ))
                dep(state["n"])
                i_colsum = bump(vector.tensor_reduce(
                    out=colsum[:, :], in_=acc[:, :],
                    axis=mybir.AxisListType.X, op=mybir.AluOpType.add,
                ))
                marks["colsum"] = i_colsum
                vector.wait_ge(sem_mm, 1)
                vector.tensor_copy(out=res[:, :], in_=tot[:, :]).then_inc(
                    sem_res, 1)

            @block.tensor
            def _(tensor):
                tensor.wait_ge(sem_dve, marks["colsum"])
                tensor.matmul(
                    out=tot[:, :], lhsT=colsum[:, :], rhs=ones[:, :],
                    start=True, stop=True,
                ).then_inc(sem_mm, 1)

    return nc


def make_inputs(losses, inputs_idx, params, cardinality):
    """Pack/shard full inputs into per-core input maps."""
    ptab = np.zeros((V, W), dtype=np.float32)
    ptab[:, :MAXC] = np.asarray(params, dtype=np.float32)
    ptab[:, MAXC] = np.asarray(cardinality).astype(np.float32)
    idx_full = np.asarray(inputs_idx, dtype=np.int32)
    losses_full = np.asarray(losses, dtype=np.float32)
    in_maps = []
    for c in range(NCORES):
        sl = slice(c * BC, (c + 1) * BC)
        in_maps.append({
            "ptab": ptab,
            "idx": np.ascontiguousarray(idx_full[sl].reshape(P, COLS)),
            "losses": np.ascontiguousarray(losses_full[sl].reshape(P, COLS * MAXC)),
        })
    return in_maps


_NC_CACHE = {}


def kernel(losses, inputs_idx, params, cardinality, trace=False, **kw):
    key = CHUNKS
    if key not in _NC_CACHE:
        _NC_CACHE[key] = build_kernel(CHUNKS)
    nc = _NC_CACHE[key]
    in_maps = make_inputs(losses, inputs_idx, params, cardinality)
    r = run_bass_kernel_spmd(nc, in_maps, list(range(NCORES)), trace=trace, **kw)
    total = np.float64(0.0)
    for c in range(NCORES):
        total += np.float64(r.results[c]["out"][0, 0])
    out = np.float32(total)
    if trace:
        kernel.last_results = r
    return np.asarray(out)


kernel.last_results = None
